# revision 30
# baseline (speedup 1.0000x reference)
"""Adaptive downsample (CARAFE-like) Trainium2 kernel, 8-core data parallel.

Reference computation (for shapes [4, 96, 256, 512] f32):
  y    = conv3x3_s2_p1(x, w1) * (gamma/sqrt(1+eps)) + beta        # [b,192,128,256]
  y    = leaky_relu(y, 0.1)
  mask = conv1x1(y, w2)                                           # [b,144,128,256]
  mask = softmax over the 9 taps within each of 16 groups
  out[c] = sum_t mask[g(c), t] * unfold(x)[c, t]                  # [b,96,128,256]

Distribution: 8 shards = (batch 4) x (output-H halves 2); halo rows are
sliced host-side so there is no inter-core communication.

Per-core layout choices:
 - channels permuted g-major (c' = 6g + i, original c = 16i + g) so the
   mask broadcast group->channels is a replication DMA with a
   partition-strided access pattern
 - x deinterleaved host-side per row into [O(258) | E(256)] sections
   (odd cols / even cols); tap kj=0 reads O[q], kj=1 reads E[q],
   kj=2 reads O[q+1] (the O section shifted one element)
 - taps 0..PETAPS-1 replicated on the tensor engine (ones matmul) with
   ACT PSUM->SBUF copies; remaining taps + recip replicated by DMA
   issued from GpSimd (SWDGE) to keep the Sync sequencer free
 - per row-pair the PE stream is software-pipelined (conv1(r),
   conv2(r-1), denom/rep(r-2)) so the PE never waits on ACT's exp
 - compute in bf16 (matmuls accumulate f32 in PSUM), output cast
   bf16->f32 on host
"""
import numpy as np
import ml_dtypes

BF = ml_dtypes.bfloat16

B, C, H, W = 4, 96, 256, 512
G, NI = 16, 6              # groups, channels per group (C = G*NI)
CO = 192                   # conv1 out channels
HO, WO = H // 2, W // 2    # 128, 256
HHALF = HO // 2            # 64 output rows per core
RPB = 4                    # row-pairs per (full) batch
ROWSEC = 514               # O(258) + E(256)
SECT = (0, 258, 1)         # kj = 0,1,2 section offsets
XROWS = 129                # input rows per core shard incl. pad row
BN_EPS = 1e-5
PETAPS = 2                 # taps replicated via tensor engine

# batch schedule: (rp_start, n_rps); last batch split to shrink the tail
BATCHES = [(0, 4), (4, 4), (8, 4), (12, 4), (16, 4), (20, 4), (24, 4),
           (28, 2), (30, 2)]

_PERM = np.array([16 * (c % NI) + (c // NI) for c in range(C)])  # c' -> orig c


def _build():
    import concourse.bass as bass
    import concourse.tile as tile
    from concourse import bacc, mybir

    nc = bacc.Bacc("TRN2", target_bir_lowering=False, debug=False, num_devices=8)
    f32, bf16 = mybir.dt.float32, mybir.dt.bfloat16

    x_ext = nc.declare_dram_parameter("x", [C, XROWS, ROWSEC], bf16, isOutput=False)
    w1l_ext = nc.declare_dram_parameter("w1l", [C, 18 * C], bf16, isOutput=False)
    w2l_ext = nc.declare_dram_parameter("w2l", [C, 288], bf16, isOutput=False)
    ones_ext = nc.declare_dram_parameter("ones", [128, 512], bf16, isOutput=False)
    onesg_ext = nc.declare_dram_parameter("onesg", [128, PETAPS * C], bf16, isOutput=False)
    bnp_ext = nc.declare_dram_parameter("bnp", [C, 2], f32, isOutput=False)
    out_ext = nc.declare_dram_parameter("out", [C, HHALF, WO], bf16, isOutput=True)

    AP = bass.AP
    mult, add = mybir.AluOpType.mult, mybir.AluOpType.add
    Lrelu, Exp = mybir.ActivationFunctionType.Prelu, mybir.ActivationFunctionType.Exp
    Copy = mybir.ActivationFunctionType.Copy

    with tile.TileContext(nc, trace_sim=False) as tc:
        with (
            tc.tile_pool(name="const", bufs=1) as cpool,
            tc.tile_pool(name="xp", bufs=3) as xpool,
            tc.tile_pool(name="yp", bufs=3) as ypool,
            tc.tile_pool(name="expp", bufs=2) as epool,
            tc.tile_pool(name="repp", bufs=1) as rpool,
            tc.tile_pool(name="stagp", bufs=2) as spool,
            tc.tile_pool(name="rcp", bufs=2) as rcpool,
            tc.tile_pool(name="outp", bufs=2) as opool,
            tc.tile_pool(name="py", bufs=2, space="PSUM") as pypool,
            tc.tile_pool(name="pmA", bufs=1, space="PSUM") as pmApool,
            tc.tile_pool(name="pmB", bufs=1, space="PSUM") as pmBpool,
            tc.tile_pool(name="pd", bufs=2, space="PSUM") as pdpool,
            tc.tile_pool(name="pr", bufs=1, space="PSUM") as prpool,
        ):
            x_first = xpool.tile([C, 17 * ROWSEC], bf16)
            nc.sync.dma_start(
                x_first[:, 0:5 * ROWSEC],
                AP(x_ext[:].tensor, 0, [[XROWS * ROWSEC, C], [1, 5 * ROWSEC]]))
            w1l = cpool.tile([C, 18 * C], bf16)
            nc.sync.dma_start(w1l[:], w1l_ext[:])
            nc.sync.dma_start(
                x_first[:, 5 * ROWSEC:17 * ROWSEC],
                AP(x_ext[:].tensor, 5 * ROWSEC, [[XROWS * ROWSEC, C], [1, 12 * ROWSEC]]))
            w2l = cpool.tile([C, 288], bf16)
            nc.sync.dma_start(w2l[:], w2l_ext[:])
            ones = cpool.tile([128, 512], bf16)
            nc.sync.dma_start(ones[:], ones_ext[:])
            onesg = cpool.tile([128, PETAPS * C], bf16)
            nc.sync.dma_start(onesg[:], onesg_ext[:])
            bnp = cpool.tile([C, 2], f32)
            nc.sync.dma_start(bnp[:], bnp_ext[:])

            def load_x(rp0, nrp):
                nrows = 4 * nrp + 1
                x_t = xpool.tile([C, 17 * ROWSEC], bf16)
                n1 = min(9, nrows)
                nc.sync.dma_start(
                    x_t[:, 0:n1 * ROWSEC],
                    AP(x_ext[:].tensor, 4 * rp0 * ROWSEC,
                       [[XROWS * ROWSEC, C], [1, n1 * ROWSEC]]),
                )
                if nrows > n1:
                    nc.sync.dma_start(
                        x_t[:, n1 * ROWSEC:nrows * ROWSEC],
                        AP(x_ext[:].tensor, (4 * rp0 + n1) * ROWSEC,
                           [[XROWS * ROWSEC, C], [1, (nrows - n1) * ROWSEC]]),
                    )
                return x_t

            x_tiles = {0: x_first, 1: load_x(*BATCHES[1])}

            for bi, (rp0, nrp) in enumerate(BATCHES):
                S = nrp * 512          # pixels per batch
                # prefetch x for batch bi+1 (issued now; starts once its
                # buffer's previous readers drain)
                if bi + 2 < len(BATCHES):
                    x_tiles[bi + 2] = load_x(*BATCHES[bi + 2])
                x_t = x_tiles.pop(bi)
                xten, xoff = x_t[:].tensor, x_t[:].offset
                xpart = list(x_t[:].ap[0])

                expA = epool.tile([128, RPB * 512], bf16)
                expB = epool.tile([16, RPB * 512], bf16)
                psum_d = pdpool.tile([64, 512], mybir.dt.float32)
                rep = rpool.tile([C, 8 * RPB * 512], bf16)
                stag = spool.tile([C, PETAPS * RPB * 512], bf16)

                ys_hist = {}
                pm_hist = {}

                def emit_conv1(r):
                    ys = []
                    for ch in range(2):
                        psum_y = pypool.tile([C, 512], mybir.dt.float32)
                        for t9 in range(9):
                            ki, kj = t9 // 3, t9 % 3
                            mv = AP(xten, xoff + (4 * r + ki) * ROWSEC + SECT[kj],
                                    [xpart, [2 * ROWSEC, 2], [1, 256]])
                            nc.tensor.matmul(
                                psum_y[:], w1l[:, (ch * 9 + t9) * C:(ch * 9 + t9 + 1) * C],
                                mv, start=(t9 == 0), stop=(t9 == 8))
                        y = ypool.tile([C, 512], bf16)
                        nc.scalar.activation(y[:], psum_y[:], Lrelu,
                                             bias=bnp[:, ch:ch + 1], scale=1.0, alpha=0.1)
                        ys.append(y)
                    ys_hist[r] = ys

                def emit_conv2(r):
                    ys = ys_hist.pop(r)
                    psum_mA = pmApool.tile([128, 512], mybir.dt.float32)
                    nc.tensor.matmul(psum_mA[:], w2l[:, 0:128], ys[0][:], start=True, stop=False)
                    nc.tensor.matmul(psum_mA[:], w2l[:, 128:256], ys[1][:], start=False, stop=True)
                    psum_mB = pmBpool.tile([16, 512], mybir.dt.float32)
                    nc.tensor.matmul(psum_mB[:], w2l[:, 256:272], ys[0][:], start=True, stop=False)
                    nc.tensor.matmul(psum_mB[:], w2l[:, 272:288], ys[1][:], start=False, stop=True)
                    nc.scalar.activation(expA[:, 512 * r:512 * (r + 1)], psum_mA[:], Exp)
                    nc.scalar.activation(expB[:, 512 * r:512 * (r + 1)], psum_mB[:], Exp)

                def emit_denom_rep(r):
                    nc.tensor.matmul(psum_d[:], ones[:, 64 * r:64 * (r + 1)],
                                     expA[:, 512 * r:512 * (r + 1)],
                                     start=(r == 0), stop=False, skip_group_check=True)
                    nc.tensor.matmul(psum_d[:], ones[0:16, 256 + 64 * r:256 + 64 * (r + 1)],
                                     expB[:, 512 * r:512 * (r + 1)],
                                     start=False, stop=(r == nrp - 1), skip_group_check=True)
                    psum_r = prpool.tile([C, PETAPS * 512], mybir.dt.float32)
                    for t in range(PETAPS):
                        nc.tensor.matmul(psum_r[:, 512 * t:512 * (t + 1)],
                                         onesg[:, C * t:C * (t + 1)],
                                         expA[:, 512 * r:512 * (r + 1)],
                                         start=True, stop=True)
                    for t in range(PETAPS):
                        nc.scalar.activation(stag[:, t * S + 512 * r:t * S + 512 * (r + 1)],
                                             psum_r[:, 512 * t:512 * (t + 1)], Copy)

                # software-pipelined emit: conv1(r) | conv2(r-1) | denom/rep(r-2)
                for r in range(nrp + 2):
                    if r < nrp:
                        emit_conv1(r)
                    if 1 <= r < nrp + 1:
                        emit_conv2(r - 1)
                    if 2 <= r:
                        emit_denom_rep(r - 2)

                # ---- 1/denom ----
                recip32 = rcpool.tile([64, 512], mybir.dt.float32)
                nc.vector.reciprocal_approx_fast(recip32[:], psum_d[:])
                recipbf = rcpool.tile([64, 512], bf16)
                nc.vector.tensor_copy(recipbf[:], recip32[:])

                # ---- replicate exp + recip group->channels: slot[6g+i] = src[g]
                #      rep slots: taps 2..8 at (t-2)*S, recip at 7*S ----
                eA = expA[:]
                pstA = list(eA.ap[0])[0]
                for t in range(PETAPS, 8):
                    nc.gpsimd.dma_start(
                        rep[:, (t - 2) * S:(t - 2) * S + S],
                        AP(eA.tensor, eA.offset + t * pstA,
                           [[8 * pstA, 16], [0, NI], [1, S]]))
                eB = expB[:]
                pstB = list(eB.ap[0])[0]
                nc.gpsimd.dma_start(
                    rep[:, 6 * S:7 * S],
                    AP(eB.tensor, eB.offset, [[pstB, 16], [0, NI], [1, S]]))
                rb = recipbf[:]
                pstR = list(rb.ap[0])[0]
                for r in range(nrp):
                    nc.gpsimd.dma_start(
                        rep[:, 7 * S + 512 * r:7 * S + 512 * (r + 1)],
                        AP(rb.tensor, rb.offset + r * pstR,
                           [[4 * pstR, 16], [0, NI], [1, 512]]))

                # ---- products (in-place): taps 0,1 in stag; 2..8 in rep ----
                for t9 in range(9):
                    ki, kj = t9 // 3, t9 % 3
                    in0 = AP(xten, xoff + ki * ROWSEC + SECT[kj],
                             [xpart, [2 * ROWSEC, 2 * nrp], [1, 256]])
                    dst = (stag[:, S * t9:S * (t9 + 1)] if t9 < PETAPS
                           else rep[:, S * (t9 - 2):S * (t9 - 1)])
                    nc.vector.tensor_tensor(dst, in0, dst, mult)

                # ---- tap-sum tree + normalize ----
                # stag: [t0|t1]; rep: [t2|t3|t4|t5|t6|t7|t8|recip]
                nc.vector.tensor_tensor(stag[:, 0:2 * S], stag[:, 0:2 * S], rep[:, 0:2 * S], add)
                nc.vector.tensor_tensor(rep[:, 2 * S:4 * S], rep[:, 2 * S:4 * S], rep[:, 4 * S:6 * S], add)
                nc.vector.tensor_tensor(stag[:, 0:2 * S], stag[:, 0:2 * S], rep[:, 2 * S:4 * S], add)
                nc.vector.tensor_tensor(stag[:, 0:S], stag[:, 0:S], stag[:, S:2 * S], add)
                nc.vector.tensor_tensor(stag[:, 0:S], stag[:, 0:S], rep[:, 6 * S:7 * S], add)
                out_t = opool.tile([C, RPB * 512], bf16)
                nc.vector.tensor_tensor(out_t[:, 0:S], stag[:, 0:S], rep[:, 7 * S:8 * S], mult)

                # ---- store (bf16; host upcasts) ----
                nc.sync.dma_start(
                    AP(out_ext[:].tensor, 2 * rp0 * WO, [[HHALF * WO, C], [1, S]]),
                    out_t[:, 0:S])

    nc.compile()
    return nc


_NC_CACHE = {}


def _get_nc():
    if "nc" not in _NC_CACHE:
        _NC_CACHE["nc"] = _build()
    return _NC_CACHE["nc"]


def _prep_weights(w1, gamma, beta, w2):
    bnscale = (gamma / np.sqrt(1.0 + BN_EPS)).astype(np.float32)
    w1s = w1.astype(np.float32) * bnscale[:, None, None, None]   # [192,96,3,3]
    # w1l[k, (ch*9+t)*96 + m] = w1s[ch*96+m, PERM[k], ki, kj]
    w1p = w1s[:, _PERM, :, :]                                    # [192,96p,3,3]
    w1l = np.zeros((C, 18 * C), np.float32)
    for ch in range(2):
        for t9 in range(9):
            ki, kj = t9 // 3, t9 % 3
            w1l[:, (ch * 9 + t9) * C:(ch * 9 + t9 + 1) * C] = \
                w1p[ch * C:(ch + 1) * C, :, ki, kj].T
    # w2l: [A0(128) | A1(128) | B0(16) | B1(16)]; A col m=8g+t (port spread),
    # B col g; orig mask channel g*9+t
    w2f = w2.astype(np.float32)[:, :, 0, 0]                      # [144,192]
    w2l = np.zeros((C, 288), np.float32)
    for t in range(8):
        for g in range(G):
            w2l[:, 8 * g + t] = w2f[g * 9 + t, 0:C]
            w2l[:, 128 + 8 * g + t] = w2f[g * 9 + t, C:2 * C]
    for g in range(G):
        w2l[:, 256 + g] = w2f[g * 9 + 8, 0:C]
        w2l[:, 272 + g] = w2f[g * 9 + 8, C:2 * C]
    # ones: denom psum partition for (rp r, group g) is 4g+r (port spread).
    # A_r block at cols 64r (rows 8g+t, t<8); B_r at cols 256+64r (rows g).
    ones = np.zeros((128, 512), np.float32)
    for r in range(4):
        for t in range(8):
            for g in range(G):
                ones[8 * g + t, 64 * r + 4 * g + r] = 1.0
        for g in range(G):
            ones[g, 256 + 64 * r + 4 * g + r] = 1.0
    # onesg: [128, PETAPS*C] replication stationary for PE-replicated taps:
    # onesg[p, t*C + c'] = 1 iff p == 8*(c'//6) + t
    onesg = np.zeros((128, PETAPS * C), np.float32)
    for t in range(PETAPS):
        for cp in range(C):
            onesg[8 * (cp // NI) + t, t * C + cp] = 1.0
    bnp = np.stack([beta[0:C], beta[C:2 * C]], axis=1).astype(np.float32)
    return (w1l.astype(BF), w2l.astype(BF), ones.astype(BF),
            onesg.astype(BF), bnp)


def _prep_x_shard(xb):
    """xb: [C, H, W] f32 already channel-permuted; returns two [C,129,514]
    bf16 shards (top half, bottom half)."""
    shards = []
    for half in range(2):
        if half == 0:
            rows = np.concatenate(
                [np.zeros((C, 1, W), np.float32), xb[:, 0:H // 2, :]], axis=1)
        else:
            rows = xb[:, H // 2 - 1:H, :]
        o = np.concatenate([np.zeros((C, XROWS, 1), np.float32),
                            rows[:, :, 1::2],
                            np.zeros((C, XROWS, 1), np.float32)], axis=2)  # O: 258
        e = rows[:, :, 0::2]                                               # E: 256
        sec = np.concatenate([o, e], axis=2)                               # 514
        shards.append(sec.astype(BF))
    return shards


def make_in_maps(x, w1, gamma, beta, w2):
    w1l, w2l, ones, onesg, bnp = _prep_weights(w1, gamma, beta, w2)
    xp = np.asarray(x)[:, _PERM, :, :].astype(np.float32)
    in_maps = []
    for b in range(B):
        halves = _prep_x_shard(xp[b])
        for half in range(2):
            in_maps.append({"x": halves[half], "w1l": w1l, "w2l": w2l,
                            "ones": ones, "onesg": onesg, "bnp": bnp})
    return in_maps


def kernel(x, w1, gamma, beta, w2):
    from concourse.bass_utils import run_bass_kernel_spmd

    nc = _get_nc()
    in_maps = make_in_maps(x, w1, gamma, beta, w2)
    res = run_bass_kernel_spmd(nc, in_maps, core_ids=list(range(8)), trace=False)

    out = np.empty((B, C, HO, WO), np.float32)
    for core in range(8):
        b, half = core // 2, core % 2
        out[b, _PERM, half * HHALF:(half + 1) * HHALF, :] = res.results[core]["out"].astype(np.float32)
    return out


# revision 34
# speedup vs baseline: 1.3087x; 1.3087x over previous
"""Adaptive downsample (CARAFE-like) Trainium2 kernel, 8-core data parallel.

Reference computation (for shapes [4, 96, 256, 512] f32):
  y    = conv3x3_s2_p1(x, w1) * (gamma/sqrt(1+eps)) + beta        # [b,192,128,256]
  y    = leaky_relu(y, 0.1)
  mask = conv1x1(y, w2)                                           # [b,144,128,256]
  mask = softmax over the 9 taps within each of 16 groups
  out[c] = sum_t mask[g(c), t] * unfold(x)[c, t]                  # [b,96,128,256]

Distribution: 8 shards = (batch 4) x (output-H halves 2); halo rows are
sliced host-side so there is no inter-core communication.

Per-core layout choices:
 - channels permuted g-major (c' = 6g + i, original c = 16i + g) so the
   mask broadcast group->channels is a replication DMA with a
   partition-strided access pattern
 - x deinterleaved host-side per row into [O(258) | E(256)] sections
   (odd cols / even cols); tap kj=0 reads O[q], kj=1 reads E[q],
   kj=2 reads O[q+1] (the O section shifted one element)
 - taps 0..PETAPS-1 replicated on the tensor engine (ones matmul) with
   ACT PSUM->SBUF copies; remaining taps + recip replicated by DMA
   issued from GpSimd (SWDGE) to keep the Sync sequencer free
 - per row-pair the PE stream is software-pipelined (conv1(r),
   conv2(r-1), denom/rep(r-2)) so the PE never waits on ACT's exp
 - compute in bf16 (matmuls accumulate f32 in PSUM), output cast
   bf16->f32 on host
"""
import numpy as np
import ml_dtypes

BF = ml_dtypes.bfloat16

B, C, H, W = 4, 96, 256, 512
G, NI = 16, 6              # groups, channels per group (C = G*NI)
CO = 192                   # conv1 out channels
HO, WO = H // 2, W // 2    # 128, 256
HHALF = HO // 2            # 64 output rows per core
RPB = 4                    # row-pairs per (full) batch
ROWSEC = 514               # O(258) + E(256)
SECT = (0, 258, 1)         # kj = 0,1,2 section offsets
XROWS = 129                # input rows per core shard incl. pad row
BN_EPS = 1e-5
PETAPS = 2                 # taps replicated via tensor engine

# batch schedule: (rp_start, n_rps); small final batches shrink the tail
BATCHES = [(0, 4), (4, 4), (8, 4), (12, 4), (16, 4), (20, 4), (24, 4),
           (28, 3), (31, 1)]

_PERM = np.array([16 * (c % NI) + (c // NI) for c in range(C)])  # c' -> orig c


def _build():
    import concourse.bass as bass
    import concourse.tile as tile
    from concourse import bacc, mybir

    nc = bacc.Bacc("TRN2", target_bir_lowering=False, debug=False, num_devices=8)
    f32, bf16 = mybir.dt.float32, mybir.dt.bfloat16

    x_ext = nc.declare_dram_parameter("x", [C, XROWS, ROWSEC], bf16, isOutput=False)
    w1l_ext = nc.declare_dram_parameter("w1l", [C, 18 * C], bf16, isOutput=False)
    w2l_ext = nc.declare_dram_parameter("w2l", [C, 288], bf16, isOutput=False)
    ones_ext = nc.declare_dram_parameter("ones", [128, 512], bf16, isOutput=False)
    onesg_ext = nc.declare_dram_parameter("onesg", [128, PETAPS * C], bf16, isOutput=False)
    bnp_ext = nc.declare_dram_parameter("bnp", [C, 2], f32, isOutput=False)
    out_ext = nc.declare_dram_parameter("out", [C, HHALF, WO], bf16, isOutput=True)

    AP = bass.AP
    mult, add = mybir.AluOpType.mult, mybir.AluOpType.add
    Lrelu, Exp = mybir.ActivationFunctionType.Prelu, mybir.ActivationFunctionType.Exp
    Copy = mybir.ActivationFunctionType.Copy

    with tile.TileContext(nc, trace_sim=False) as tc:
        with (
            tc.tile_pool(name="const", bufs=1) as cpool,
            tc.tile_pool(name="xp", bufs=2) as xpool,
            tc.tile_pool(name="yp", bufs=3) as ypool,
            tc.tile_pool(name="expp", bufs=2) as epool,
            tc.tile_pool(name="repp", bufs=2) as rpool,
            tc.tile_pool(name="stagp", bufs=2) as spool,
            tc.tile_pool(name="rcp", bufs=2) as rcpool,
            tc.tile_pool(name="outp", bufs=2) as opool,
            tc.tile_pool(name="py", bufs=2, space="PSUM") as pypool,
            tc.tile_pool(name="pmA", bufs=1, space="PSUM") as pmApool,
            tc.tile_pool(name="pmB", bufs=1, space="PSUM") as pmBpool,
            tc.tile_pool(name="pd", bufs=2, space="PSUM") as pdpool,
            tc.tile_pool(name="pr", bufs=1, space="PSUM") as prpool,
        ):
            x_first = xpool.tile([C, 17 * ROWSEC], bf16)
            nc.sync.dma_start(
                x_first[:, 0:5 * ROWSEC],
                AP(x_ext[:].tensor, 0, [[XROWS * ROWSEC, C], [1, 5 * ROWSEC]]))
            w1l = cpool.tile([C, 18 * C], bf16)
            nc.sync.dma_start(w1l[:], w1l_ext[:])
            nc.sync.dma_start(
                x_first[:, 5 * ROWSEC:17 * ROWSEC],
                AP(x_ext[:].tensor, 5 * ROWSEC, [[XROWS * ROWSEC, C], [1, 12 * ROWSEC]]))
            w2l = cpool.tile([C, 288], bf16)
            nc.sync.dma_start(w2l[:], w2l_ext[:])
            ones = cpool.tile([128, 512], bf16)
            nc.sync.dma_start(ones[:], ones_ext[:])
            onesg = cpool.tile([128, PETAPS * C], bf16)
            nc.sync.dma_start(onesg[:], onesg_ext[:])
            bnp = cpool.tile([C, 2], f32)
            nc.sync.dma_start(bnp[:], bnp_ext[:])

            def load_x(rp0, nrp):
                nrows = 4 * nrp + 1
                x_t = xpool.tile([C, 17 * ROWSEC], bf16)
                # chunks: rows [0,5) -> rp0; [5,9) -> rp1; [9,nrows) -> rest
                row_chunks = [(0, min(5, nrows)), (5, min(9, nrows)), (9, nrows)]
                for (a, b) in row_chunks:
                    if b <= a:
                        continue
                    nc.sync.dma_start(
                        x_t[:, a * ROWSEC:b * ROWSEC],
                        AP(x_ext[:].tensor, (4 * rp0 + a) * ROWSEC,
                           [[XROWS * ROWSEC, C], [1, (b - a) * ROWSEC]]),
                    )
                return x_t

            x_tiles = {0: x_first, 1: load_x(*BATCHES[1])}

            for bi, (rp0, nrp) in enumerate(BATCHES):
                S = nrp * 512          # pixels per batch
                # prefetch x for batch bi+1 (issued now; starts once its
                # buffer's previous readers drain)
                if bi + 2 < len(BATCHES):
                    x_tiles[bi + 2] = load_x(*BATCHES[bi + 2])
                x_t = x_tiles.pop(bi)
                xten, xoff = x_t[:].tensor, x_t[:].offset
                xpart = list(x_t[:].ap[0])

                expA = epool.tile([128, RPB * 512], bf16)
                expB = epool.tile([16, RPB * 512], bf16)
                psum_d = pdpool.tile([64, 512], mybir.dt.float32)
                rep = rpool.tile([C, 8 * RPB * 512], bf16)
                stag = spool.tile([C, PETAPS * RPB * 512], bf16)

                ys_hist = {}
                pm_hist = {}

                def emit_conv1(r):
                    ys = []
                    for ch in range(2):
                        psum_y = pypool.tile([C, 512], mybir.dt.float32)
                        for t9 in range(9):
                            ki, kj = t9 // 3, t9 % 3
                            mv = AP(xten, xoff + (4 * r + ki) * ROWSEC + SECT[kj],
                                    [xpart, [2 * ROWSEC, 2], [1, 256]])
                            nc.tensor.matmul(
                                psum_y[:], w1l[:, (ch * 9 + t9) * C:(ch * 9 + t9 + 1) * C],
                                mv, start=(t9 == 0), stop=(t9 == 8))
                        y = ypool.tile([C, 512], bf16)
                        nc.scalar.activation(y[:], psum_y[:], Lrelu,
                                             bias=bnp[:, ch:ch + 1], scale=1.0, alpha=0.1)
                        ys.append(y)
                    ys_hist[r] = ys

                def emit_conv2(r):
                    ys = ys_hist.pop(r)
                    psum_mA = pmApool.tile([128, 512], mybir.dt.float32)
                    nc.tensor.matmul(psum_mA[:], w2l[:, 0:128], ys[0][:], start=True, stop=False)
                    nc.tensor.matmul(psum_mA[:], w2l[:, 128:256], ys[1][:], start=False, stop=True)
                    psum_mB = pmBpool.tile([16, 512], mybir.dt.float32)
                    nc.tensor.matmul(psum_mB[:], w2l[:, 256:272], ys[0][:], start=True, stop=False)
                    nc.tensor.matmul(psum_mB[:], w2l[:, 272:288], ys[1][:], start=False, stop=True)
                    nc.scalar.activation(expA[:, 512 * r:512 * (r + 1)], psum_mA[:], Exp)
                    nc.scalar.activation(expB[:, 512 * r:512 * (r + 1)], psum_mB[:], Exp)

                def emit_denom_rep(r):
                    nc.tensor.matmul(psum_d[:], ones[:, 64 * r:64 * (r + 1)],
                                     expA[:, 512 * r:512 * (r + 1)],
                                     start=(r == 0), stop=False, skip_group_check=True)
                    nc.tensor.matmul(psum_d[:], ones[0:16, 256 + 64 * r:256 + 64 * (r + 1)],
                                     expB[:, 512 * r:512 * (r + 1)],
                                     start=False, stop=(r == nrp - 1), skip_group_check=True)
                    psum_r = prpool.tile([C, PETAPS * 512], mybir.dt.float32)
                    for t in range(PETAPS):
                        nc.tensor.matmul(psum_r[:, 512 * t:512 * (t + 1)],
                                         onesg[:, C * t:C * (t + 1)],
                                         expA[:, 512 * r:512 * (r + 1)],
                                         start=True, stop=True)
                    for t in range(PETAPS):
                        nc.scalar.activation(stag[:, t * S + 512 * r:t * S + 512 * (r + 1)],
                                             psum_r[:, 512 * t:512 * (t + 1)], Copy)

                # software-pipelined emit: conv1(r) | conv2(r-1) | denom/rep(r-2)
                for r in range(nrp + 2):
                    if r < nrp:
                        emit_conv1(r)
                    if 1 <= r < nrp + 1:
                        emit_conv2(r - 1)
                    if 2 <= r:
                        emit_denom_rep(r - 2)

                # ---- 1/denom ----
                recip32 = rcpool.tile([64, 512], mybir.dt.float32)
                nc.vector.reciprocal_approx_fast(recip32[:], psum_d[:])
                recipbf = rcpool.tile([64, 512], bf16)
                nc.vector.tensor_copy(recipbf[:], recip32[:])

                # ---- replicate exp + recip group->channels: slot[6g+i] = src[g]
                #      rep slots: taps 2..8 at (t-2)*S, recip at 7*S ----
                eA = expA[:]
                pstA = list(eA.ap[0])[0]
                for t in range(PETAPS, 8):
                    nc.gpsimd.dma_start(
                        rep[:, (t - 2) * S:(t - 2) * S + S],
                        AP(eA.tensor, eA.offset + t * pstA,
                           [[8 * pstA, 16], [0, NI], [1, S]]))
                eB = expB[:]
                pstB = list(eB.ap[0])[0]
                nc.gpsimd.dma_start(
                    rep[:, 6 * S:7 * S],
                    AP(eB.tensor, eB.offset, [[pstB, 16], [0, NI], [1, S]]))
                rb = recipbf[:]
                pstR = list(rb.ap[0])[0]
                for r in range(nrp):
                    nc.gpsimd.dma_start(
                        rep[:, 7 * S + 512 * r:7 * S + 512 * (r + 1)],
                        AP(rb.tensor, rb.offset + r * pstR,
                           [[4 * pstR, 16], [0, NI], [1, 512]]))

                # ---- products (in-place): taps 0,1 in stag; 2..8 in rep ----
                for t9 in range(9):
                    ki, kj = t9 // 3, t9 % 3
                    in0 = AP(xten, xoff + ki * ROWSEC + SECT[kj],
                             [xpart, [2 * ROWSEC, 2 * nrp], [1, 256]])
                    dst = (stag[:, S * t9:S * (t9 + 1)] if t9 < PETAPS
                           else rep[:, S * (t9 - 2):S * (t9 - 1)])
                    nc.vector.tensor_tensor(dst, in0, dst, mult)

                # ---- tap-sum tree + normalize ----
                # stag: [t0|t1]; rep: [t2|t3|t4|t5|t6|t7|t8|recip]
                nc.vector.tensor_tensor(stag[:, 0:2 * S], stag[:, 0:2 * S], rep[:, 0:2 * S], add)
                nc.vector.tensor_tensor(rep[:, 2 * S:4 * S], rep[:, 2 * S:4 * S], rep[:, 4 * S:6 * S], add)
                nc.vector.tensor_tensor(stag[:, 0:2 * S], stag[:, 0:2 * S], rep[:, 2 * S:4 * S], add)
                nc.vector.tensor_tensor(stag[:, 0:S], stag[:, 0:S], stag[:, S:2 * S], add)
                nc.vector.tensor_tensor(stag[:, 0:S], stag[:, 0:S], rep[:, 6 * S:7 * S], add)
                out_t = opool.tile([C, RPB * 512], bf16)
                nc.vector.tensor_tensor(out_t[:, 0:S], stag[:, 0:S], rep[:, 7 * S:8 * S], mult)

                # ---- store (bf16; host upcasts) ----
                nc.sync.dma_start(
                    AP(out_ext[:].tensor, 2 * rp0 * WO, [[HHALF * WO, C], [1, S]]),
                    out_t[:, 0:S])

    nc.compile()
    return nc


_NC_CACHE = {}


def _get_nc():
    if "nc" not in _NC_CACHE:
        _NC_CACHE["nc"] = _build()
    return _NC_CACHE["nc"]


def _prep_weights(w1, gamma, beta, w2):
    bnscale = (gamma / np.sqrt(1.0 + BN_EPS)).astype(np.float32)
    w1s = w1.astype(np.float32) * bnscale[:, None, None, None]   # [192,96,3,3]
    # w1l[k, (ch*9+t)*96 + m] = w1s[ch*96+m, PERM[k], ki, kj]
    w1p = w1s[:, _PERM, :, :]                                    # [192,96p,3,3]
    w1l = np.zeros((C, 18 * C), np.float32)
    for ch in range(2):
        for t9 in range(9):
            ki, kj = t9 // 3, t9 % 3
            w1l[:, (ch * 9 + t9) * C:(ch * 9 + t9 + 1) * C] = \
                w1p[ch * C:(ch + 1) * C, :, ki, kj].T
    # w2l: [A0(128) | A1(128) | B0(16) | B1(16)]; A col m=8g+t (port spread),
    # B col g; orig mask channel g*9+t
    w2f = w2.astype(np.float32)[:, :, 0, 0]                      # [144,192]
    w2l = np.zeros((C, 288), np.float32)
    for t in range(8):
        for g in range(G):
            w2l[:, 8 * g + t] = w2f[g * 9 + t, 0:C]
            w2l[:, 128 + 8 * g + t] = w2f[g * 9 + t, C:2 * C]
    for g in range(G):
        w2l[:, 256 + g] = w2f[g * 9 + 8, 0:C]
        w2l[:, 272 + g] = w2f[g * 9 + 8, C:2 * C]
    # ones: denom psum partition for (rp r, group g) is 4g+r (port spread).
    # A_r block at cols 64r (rows 8g+t, t<8); B_r at cols 256+64r (rows g).
    ones = np.zeros((128, 512), np.float32)
    for r in range(4):
        for t in range(8):
            for g in range(G):
                ones[8 * g + t, 64 * r + 4 * g + r] = 1.0
        for g in range(G):
            ones[g, 256 + 64 * r + 4 * g + r] = 1.0
    # onesg: [128, PETAPS*C] replication stationary for PE-replicated taps:
    # onesg[p, t*C + c'] = 1 iff p == 8*(c'//6) + t
    onesg = np.zeros((128, PETAPS * C), np.float32)
    for t in range(PETAPS):
        for cp in range(C):
            onesg[8 * (cp // NI) + t, t * C + cp] = 1.0
    bnp = np.stack([beta[0:C], beta[C:2 * C]], axis=1).astype(np.float32)
    return (w1l.astype(BF), w2l.astype(BF), ones.astype(BF),
            onesg.astype(BF), bnp)


def _prep_x_shard(xb):
    """xb: [C, H, W] f32 already channel-permuted; returns two [C,129,514]
    bf16 shards (top half, bottom half)."""
    shards = []
    for half in range(2):
        if half == 0:
            rows = np.concatenate(
                [np.zeros((C, 1, W), np.float32), xb[:, 0:H // 2, :]], axis=1)
        else:
            rows = xb[:, H // 2 - 1:H, :]
        o = np.concatenate([np.zeros((C, XROWS, 1), np.float32),
                            rows[:, :, 1::2],
                            np.zeros((C, XROWS, 1), np.float32)], axis=2)  # O: 258
        e = rows[:, :, 0::2]                                               # E: 256
        sec = np.concatenate([o, e], axis=2)                               # 514
        shards.append(sec.astype(BF))
    return shards


def make_in_maps(x, w1, gamma, beta, w2):
    w1l, w2l, ones, onesg, bnp = _prep_weights(w1, gamma, beta, w2)
    xp = np.asarray(x)[:, _PERM, :, :].astype(np.float32)
    in_maps = []
    for b in range(B):
        halves = _prep_x_shard(xp[b])
        for half in range(2):
            in_maps.append({"x": halves[half], "w1l": w1l, "w2l": w2l,
                            "ones": ones, "onesg": onesg, "bnp": bnp})
    return in_maps


def kernel(x, w1, gamma, beta, w2):
    from concourse.bass_utils import run_bass_kernel_spmd

    nc = _get_nc()
    in_maps = make_in_maps(x, w1, gamma, beta, w2)
    res = run_bass_kernel_spmd(nc, in_maps, core_ids=list(range(8)), trace=False)

    out = np.empty((B, C, HO, WO), np.float32)
    for core in range(8):
        b, half = core // 2, core % 2
        out[b, _PERM, half * HHALF:(half + 1) * HHALF, :] = res.results[core]["out"].astype(np.float32)
    return out
